# revision 1
# baseline (speedup 1.0000x reference)
"""L21 norm kernel for Trainium2 (Bass/Tile), 8-core SPMD.

Computes sum_j sqrt(sum_i S[i,j]^2) for S of shape [8192, 16384] fp32.

Sharding: S is split along columns into 8 shards of [8192, 2048] (one per
NeuronCore). Each core computes the sum of its columns' L2 norms as a
scalar; the host sums the 8 partial scalars.

Per-core dataflow (memory-bound; ~64 MiB HBM read per core):
  - 16 macro tiles of [128 partitions, 4 row-blocks, 2048 cols] fp32
    (4 MiB per HWDGE DMA).
  - ACT engine: square with bf16 output (also the dtype cast for PE).
  - PE: ones[128,1]^T @ sq[128,512] matmuls reduce the partition axis,
    accumulating per-column sums of squares into PSUM [1,2048] fp32
    across all 64 row-blocks.
  - Epilogue: ACT sqrt (PSUM -> SBUF), DVE free-axis reduce_sum -> [1,1],
    DMA to DRAM.
"""

import numpy as np

# Full problem shape (hardcoded per the harness contract).
R = 8192          # rows
C_FULL = 16384    # columns
N_CORES = 8
C = C_FULL // N_CORES  # 2048 columns per core
P = 128           # SBUF partitions
G = 4             # row-blocks per macro tile
T = R // (P * G)  # macro tiles per core
NBLK = 512        # matmul moving free dim (one PSUM bank of fp32)

_cached = None


def _build():
    """Build + schedule the per-core Bass program. Returns the Bacc object."""
    import concourse.bacc as bacc
    import concourse.tile as tile
    from concourse import mybir

    nc = bacc.Bacc(
        "TRN2",
        target_bir_lowering=False,
        debug=False,
        enable_asserts=False,
        num_devices=N_CORES,
    )

    s_dram = nc.dram_tensor("S", [R, C], mybir.dt.float32, kind="ExternalInput")
    out_dram = nc.dram_tensor("out", [1, 1], mybir.dt.float32, kind="ExternalOutput")

    s_ap = s_dram.ap()
    out_ap = out_dram.ap()

    # [T, P, G, C]: macro tile t covers rows [t*G*P, (t+1)*G*P)
    s_view = s_ap.rearrange("(t g p) c -> t p g c", p=P, g=G)

    with tile.TileContext(nc) as tc:
        with (
            tc.tile_pool(name="io", bufs=3) as io_pool,
            tc.tile_pool(name="sqp", bufs=2) as sq_pool,
            tc.tile_pool(name="const", bufs=1) as const_pool,
            tc.tile_pool(name="ps", bufs=1, space="PSUM") as ps_pool,
            tc.tile_pool(name="fin", bufs=1) as fin_pool,
        ):
            ones = const_pool.tile([P, 1], mybir.dt.bfloat16)
            nc.vector.memset(ones, 1.0)

            # Per-column sum of squares accumulator (4 PSUM banks).
            colsq = ps_pool.tile([1, C], mybir.dt.float32)

            for t in range(T):
                x_tile = io_pool.tile([P, G, C], mybir.dt.float32, tag="x")
                nc.sync.dma_start(out=x_tile, in_=s_view[t])

                sq = sq_pool.tile([P, G, C], mybir.dt.bfloat16, tag="sq")
                nc.scalar.square(out=sq, in_=x_tile)

                for g in range(G):
                    for b in range(C // NBLK):
                        nc.tensor.matmul(
                            colsq[:, b * NBLK : (b + 1) * NBLK],
                            ones,
                            sq[:, g, b * NBLK : (b + 1) * NBLK],
                            start=(t == 0 and g == 0),
                            stop=(t == T - 1 and g == G - 1),
                        )

            norms = fin_pool.tile([1, C], mybir.dt.float32)
            nc.scalar.sqrt(out=norms, in_=colsq)

            total = fin_pool.tile([1, 1], mybir.dt.float32)
            nc.vector.reduce_sum(out=total, in_=norms, axis=mybir.AxisListType.X)

            nc.sync.dma_start(out=out_ap, in_=total)

    nc.compile()
    return nc


def _get_nc():
    global _cached
    if _cached is None:
        _cached = _build()
    return _cached


def _run(S: np.ndarray, trace: bool = False):
    from concourse import bass_utils

    assert S.shape == (R, C_FULL), S.shape
    S = np.ascontiguousarray(np.asarray(S, dtype=np.float32))

    nc = _get_nc()
    in_maps = [
        {"S": np.ascontiguousarray(S[:, i * C : (i + 1) * C])} for i in range(N_CORES)
    ]
    res = bass_utils.run_bass_kernel_spmd(
        nc, in_maps, core_ids=list(range(N_CORES)), trace=trace
    )
    partials = np.array(
        [res.results[i]["out"][0, 0] for i in range(N_CORES)], dtype=np.float64
    )
    out = np.float32(partials.sum())
    return out, res


def kernel(S: np.ndarray) -> np.ndarray:
    out, _ = _run(S, trace=False)
    return np.asarray(out, dtype=np.float32)


def run_traced(S: np.ndarray):
    """For test.py: returns (output, BassKernelResults) with NTFF trace."""
    return _run(S, trace=True)


# revision 2
# speedup vs baseline: 1.2287x; 1.2287x over previous
"""L21 norm kernel for Trainium2 (Bass/Tile), 8-core SPMD.

Computes sum_j sqrt(sum_i S[i,j]^2) for S of shape [8192, 16384] fp32.

Sharding: S is split along columns into 8 shards of [8192, 2048] (one per
NeuronCore). Each core computes the sum of its columns' L2 norms as a
scalar; the host sums the 8 partial scalars.

Per-core dataflow (memory-bound; 64 MiB HBM read per core, ~187 us floor
at 358 GB/s per-NC):
  - 32 tiles of [128 partitions, 2 rows, 2048 cols] fp32 (2 MiB HWDGE
    DMAs; each partition's slice is 16 KiB contiguous in DRAM).
  - ACT engine: square with bf16 output (also the dtype cast for PE).
  - Partition-axis reduction is split so neither engine paces the DMA
    stream: per tile, row-slice q=0 goes to PE (ones[128,1]^T @ sq
    matmuls accumulating into PSUM [1,2048] fp32) and row-slice q=1 is
    accumulated on DVE into a bf16 [128,2048] accumulator (2x mode).
    The DVE accumulator is folded into PSUM via 4 matmuls near the end;
    the last tile sends both row-slices to PE to keep the tail short.
  - Epilogue: ACT sqrt (PSUM -> SBUF), DVE free-axis reduce_sum -> [1,1],
    DMA to DRAM.
"""

import numpy as np

# Full problem shape (hardcoded per the harness contract).
R = 8192          # rows
C_FULL = 16384    # columns
N_CORES = 8
C = C_FULL // N_CORES  # 2048 columns per core
P = 128           # SBUF partitions
Q = 2             # rows per partition per tile (16 KiB contiguous DRAM)
T = R // (P * Q)  # tiles per core (32)
NBLK = 512        # matmul moving free dim (one PSUM bank of fp32)

_cached = None


def _build():
    """Build + schedule the per-core Bass program. Returns the Bacc object."""
    import concourse.bacc as bacc
    import concourse.tile as tile
    from concourse import mybir

    nc = bacc.Bacc(
        "TRN2",
        target_bir_lowering=False,
        debug=False,
        enable_asserts=False,
        num_devices=N_CORES,
    )

    s_dram = nc.dram_tensor("S", [R, C], mybir.dt.float32, kind="ExternalInput")
    out_dram = nc.dram_tensor("out", [1, 1], mybir.dt.float32, kind="ExternalOutput")

    s_ap = s_dram.ap()
    out_ap = out_dram.ap()

    # [T, P, Q, C]: tile t covers rows [t*P*Q, (t+1)*P*Q); partition p holds
    # Q consecutive rows -> 16 KiB contiguous DRAM per (t, p) descriptor.
    s_view = s_ap.rearrange("(t p q) c -> t p q c", p=P, q=Q)

    with tile.TileContext(nc) as tc:
        with (
            tc.tile_pool(name="io", bufs=6) as io_pool,
            tc.tile_pool(name="sqp", bufs=3) as sq_pool,
            tc.tile_pool(name="const", bufs=1) as const_pool,
            tc.tile_pool(name="ps", bufs=1, space="PSUM") as ps_pool,
            tc.tile_pool(name="fin", bufs=1) as fin_pool,
        ):
            ones = const_pool.tile([P, 1], mybir.dt.bfloat16)
            nc.vector.memset(ones, 1.0)

            # DVE-side accumulator for q=1 row-slices.
            acc = const_pool.tile([P, C], mybir.dt.bfloat16)

            # Per-column sum of squares (4 PSUM banks).
            colsq = ps_pool.tile([1, C], mybir.dt.float32)

            def pe_reduce(src, first, last):
                for b in range(C // NBLK):
                    nc.tensor.matmul(
                        colsq[:, b * NBLK : (b + 1) * NBLK],
                        ones,
                        src[:, b * NBLK : (b + 1) * NBLK],
                        start=first,
                        stop=last,
                    )

            for t in range(T):
                x_tile = io_pool.tile([P, Q, C], mybir.dt.float32, tag="x")
                nc.sync.dma_start(out=x_tile, in_=s_view[t])

                sq = sq_pool.tile([P, Q, C], mybir.dt.bfloat16, tag="sq")
                nc.scalar.square(out=sq, in_=x_tile)

                # q=0 row-slice -> PE psum accumulate.
                pe_reduce(sq[:, 0, :], first=(t == 0), last=False)

                # q=1 row-slice -> DVE bf16 accumulator (last tile -> PE,
                # so the tail doesn't need another accumulator fold).
                if t == 0:
                    nc.vector.tensor_copy(acc, sq[:, 1, :])
                elif t < T - 1:
                    nc.vector.tensor_add(acc, acc, sq[:, 1, :])
                else:
                    pe_reduce(sq[:, 1, :], first=False, last=True)

                # Fold the DVE accumulator into PSUM while the last tile's
                # DMA is still in flight.
                if t == T - 2:
                    pe_reduce(acc, first=False, last=False)

            norms = fin_pool.tile([1, C], mybir.dt.float32)
            nc.scalar.sqrt(out=norms, in_=colsq)

            total = fin_pool.tile([1, 1], mybir.dt.float32)
            nc.vector.reduce_sum(out=total, in_=norms, axis=mybir.AxisListType.X)

            nc.sync.dma_start(out=out_ap, in_=total)

    nc.compile()
    return nc


def _get_nc():
    global _cached
    if _cached is None:
        _cached = _build()
    return _cached


def _run(S: np.ndarray, trace: bool = False):
    from concourse import bass_utils

    assert S.shape == (R, C_FULL), S.shape
    S = np.ascontiguousarray(np.asarray(S, dtype=np.float32))

    nc = _get_nc()
    in_maps = [
        {"S": np.ascontiguousarray(S[:, i * C : (i + 1) * C])} for i in range(N_CORES)
    ]
    res = bass_utils.run_bass_kernel_spmd(
        nc, in_maps, core_ids=list(range(N_CORES)), trace=trace
    )
    partials = np.array(
        [res.results[i]["out"][0, 0] for i in range(N_CORES)], dtype=np.float64
    )
    out = np.float32(partials.sum())
    return out, res


def kernel(S: np.ndarray) -> np.ndarray:
    out, _ = _run(S, trace=False)
    return np.asarray(out, dtype=np.float32)


def run_traced(S: np.ndarray):
    """For test.py: returns (output, BassKernelResults) with NTFF trace."""
    return _run(S, trace=True)
